# revision 4
# baseline (speedup 1.0000x reference)
"""Trainium2 Bass kernel for nn_BoardEncoder (HexConv board encoder).

Math:
  h[b,n,:] = relu(x[b,n] @ Wc.T + sum_k neighbors[b,n,k] @ Wd[k].T + bc + bd.sum(0))
  out[b]   = h[b].reshape(216) @ Wf.T + bf

Strategy (pure data-parallel over batch, 8 cores x 2048 rows):
  - Host packs per-(b,n) token features [x | neighbors | 1.0] into a
    feature-major bf16 layout xt[n, p, c*2048 + b] = feat[c*113 + p] so the
    device streams big contiguous DMAs and the PE contracts over the
    feature dim (features on partitions), K split into 4 chunks of 113.
    bf16 halves HBM traffic (tolerance is 2e-2; bf16 lands ~3e-3) and runs
    the PE at 1 cycle/row instead of fp32's 4.
  - Input loads are whole-tile [113, 16KB] dma_starts on the sync HWDGE
    ring: HWDGE assigns descriptors round-robin across all 16 SDMA engines
    (~25 GB/s each), so a single queue sustains ~350+ GB/s. (The SWDGE
    gpsimd path concentrates a partition-sliced load on ~3 engines --
    measured 95 GB/s aggregate -- so it is NOT used for the big loads.)
  - Stage 1 (per board cell n): psum[4, 512b] += Wchunk.T @ xtchunk over 4
    chunks, relu (vector/gpsimd alternate) -> bf16 strip [4, 2048], then a
    SBUF->SBUF DMA on the scalar HWDGE ring scatters the strip to partition
    4n of the h^T [(n,h), b] accumulator.
  - Stage 2: out[128b, 256] = hA.T @ WfT[:128] + hB.T @ WfT[128:] with a
    constant ones-row in hB providing the bf bias.
"""

import sys

sys.path.insert(0, "/opt/trn_rl_repo")

import numpy as np

B = 16384
N = 54
D_IN = 64
KN = 6
D_HID = 4
D_OUT = 256
NCORES = 8
BS = B // NCORES          # 2048 batch rows per core
F = D_IN + KN * D_IN + 1  # 449 features incl. constant-1 bias feature
CH = 113                  # K-chunk partition size (4 * 113 = 452 >= 449)
NCH = 4
FPAD = CH * NCH           # 452
BT = 512                  # stage-1 moving free dim (tokens per matmul)
NBT = BS // BT            # 4

LAST_EXEC_NS = None

_PROGRAM = None


def _build_program(reps=1):
    import concourse.bacc as bacc
    import concourse.tile as tile
    from concourse import mybir

    f32 = mybir.dt.float32
    bf16 = mybir.dt.bfloat16

    nc = bacc.Bacc("TRN2", target_bir_lowering=False, debug=False,
                   num_devices=NCORES)
    xt_d = nc.declare_dram_parameter("xt", [N, CH, NCH * BS], bf16,
                                     isOutput=False)
    w_d = nc.declare_dram_parameter("w", [CH, NCH * D_HID], bf16,
                                    isOutput=False)
    wfta_d = nc.declare_dram_parameter("wfta", [128, D_OUT], bf16,
                                       isOutput=False)
    wftb_d = nc.declare_dram_parameter("wftb", [89, D_OUT], bf16,
                                       isOutput=False)
    out_d = nc.declare_dram_parameter("out", [BS, D_OUT], f32, isOutput=True)

    with tile.TileContext(nc) as tc:
        with (
            tc.tile_pool(name="consts", bufs=1) as consts,
            tc.tile_pool(name="hacc", bufs=1) as hacc,
            tc.tile_pool(name="xt", bufs=6) as xtp,
            tc.tile_pool(name="hn", bufs=3) as hnp,
            tc.tile_pool(name="ps1", bufs=4, space="PSUM") as ps1,
            tc.tile_pool(name="ps2", bufs=2, space="PSUM") as ps2,
            tc.tile_pool(name="outp", bufs=3) as outp,
        ):
            w_sb = consts.tile([CH, NCH * D_HID], bf16, tag="w")
            nc.sync.dma_start(w_sb[:], w_d[:])
            wfta_sb = consts.tile([128, D_OUT], bf16, tag="wfta")
            nc.sync.dma_start(wfta_sb[:], wfta_d[:])
            wftb_sb = consts.tile([89, D_OUT], bf16, tag="wftb")
            nc.sync.dma_start(wftb_sb[:], wftb_d[:])

            for rep in range(reps):
                hA = hacc.tile([128, BS], bf16, tag="hA")  # (n,h) rows 0..127
                hB = hacc.tile([89, BS], bf16, tag="hB")   # rows 128..215+ones
                # rows 0..87 are overwritten by the per-cell scatter DMAs
                # below; row 88 keeps the 1.0 fill and provides the bf bias
                # in stage 2. (a [88:89] memset is rejected: compute-engine
                # partition bases must be 32-aligned)
                nc.gpsimd.memset(hB[:, :], 1.0)

                for n in range(N):
                    xt = xtp.tile([CH, NCH * BS], bf16)
                    # whole-tile load on the sync HWDGE ring: 113 descriptors
                    # of 16 KB round-robin across all 16 SDMA engines
                    nc.sync.dma_start(xt[:], xt_d[n])
                    hn = hnp.tile([D_HID, BS], bf16)
                    for bt in range(NBT):
                        ps = ps1.tile([D_HID, BT], f32)
                        for c in range(NCH):
                            nc.tensor.matmul(
                                ps[:],
                                w_sb[:, c * D_HID:(c + 1) * D_HID],
                                xt[:, c * BS + bt * BT:
                                   c * BS + (bt + 1) * BT],
                                start=(c == 0),
                                stop=(c == NCH - 1),
                            )
                        dst = hn[:, bt * BT:(bt + 1) * BT]
                        if (n * NBT + bt) % 2 == 0:
                            nc.vector.tensor_scalar_max(dst, ps[:], 0.0)
                        else:
                            nc.scalar.activation(
                                dst, ps[:],
                                mybir.ActivationFunctionType.Relu)
                    # scatter on gpsimd/SWDGE: small (16 KB) and keeps its
                    # relu sem-wait off both HWDGE rings (loads on sync,
                    # stores on scalar)
                    if n < 32:
                        nc.gpsimd.dma_start(hA[n * 4:(n + 1) * 4, :], hn[:])
                    else:
                        m = n - 32
                        nc.gpsimd.dma_start(hB[m * 4:(m + 1) * 4, :], hn[:])

                for t in range(BS // 128):
                    po = ps2.tile([128, D_OUT], f32)
                    nc.tensor.matmul(po[:], hA[:, t * 128:(t + 1) * 128],
                                     wfta_sb[:], start=True, stop=False)
                    nc.tensor.matmul(po[:], hB[:, t * 128:(t + 1) * 128],
                                     wftb_sb[:], start=False, stop=True)
                    ot = outp.tile([128, D_OUT], f32)
                    if t % 2 == 0:
                        nc.vector.tensor_copy(ot[:], po[:])
                    else:
                        nc.scalar.copy(ot[:], po[:])
                    nc.scalar.dma_start(out_d[t * 128:(t + 1) * 128, :], ot[:])

    nc.compile()
    return nc


def _get_program():
    global _PROGRAM
    if _PROGRAM is None:
        _PROGRAM = _build_program()
    return _PROGRAM


def _pack_inputs(x, neighbors):
    """Per-shard feature-major packing: xt[n, p, c*BS + b] = feat[c*113+p]
    of batch row (shard*BS + b), cell n. feat = [x | neighbors | 1 | 0pad],
    cast to bf16."""
    import ml_dtypes

    bf16 = np.dtype(ml_dtypes.bfloat16)
    xts = []
    feat = np.empty((BS, N, FPAD), np.float32)
    feat[:, :, F - 1] = 1.0
    feat[:, :, F:] = 0.0
    for s in range(NCORES):
        sl = slice(s * BS, (s + 1) * BS)
        feat[:, :, :D_IN] = x[sl]
        feat[:, :, D_IN:F - 1] = neighbors[sl].reshape(BS, N, KN * D_IN)
        fb = feat.astype(bf16)                       # [BS, N, 452]
        fv = fb.reshape(BS, N, NCH, CH)
        xt = np.ascontiguousarray(fv.transpose(1, 3, 2, 0)).reshape(
            N, CH, NCH * BS)
        xts.append(xt)
    return xts


def _pack_weights(Wc, bc, Wd, bd, Wf, bf):
    import ml_dtypes

    bf16 = np.dtype(ml_dtypes.bfloat16)
    W_all = np.zeros((FPAD, D_HID), np.float32)
    W_all[:D_IN] = Wc.T
    W_all[D_IN:F - 1] = Wd.transpose(0, 2, 1).reshape(KN * D_IN, D_HID)
    W_all[F - 1] = bc + bd.sum(0)
    # w[p, c*4+h] = W_all[c*113+p, h]
    w = np.ascontiguousarray(
        W_all.reshape(NCH, CH, D_HID).transpose(1, 0, 2)).reshape(
            CH, NCH * D_HID).astype(bf16)
    WfT = np.ascontiguousarray(Wf.T)            # [216, 256]
    wfta = np.ascontiguousarray(WfT[:128]).astype(bf16)
    wftb = np.concatenate([WfT[128:], bf[None, :]], axis=0)  # [89, 256]
    wftb = np.ascontiguousarray(wftb).astype(bf16)
    return w, wfta, wftb


def kernel(x, neighbors, Wc, bc, Wd, bd, Wf, bf):
    global LAST_EXEC_NS
    from concourse.bass_utils import run_bass_kernel_spmd

    x = np.asarray(x, np.float32)
    neighbors = np.asarray(neighbors, np.float32)
    w, wfta, wftb = _pack_weights(
        np.asarray(Wc, np.float32), np.asarray(bc, np.float32),
        np.asarray(Wd, np.float32), np.asarray(bd, np.float32),
        np.asarray(Wf, np.float32), np.asarray(bf, np.float32))
    xts = _pack_inputs(x, neighbors)

    nc = _get_program()
    in_maps = [
        {"xt": xts[s], "w": w, "wfta": wfta, "wftb": wftb}
        for s in range(NCORES)
    ]
    res = run_bass_kernel_spmd(nc, in_maps, list(range(NCORES)))
    LAST_EXEC_NS = res.exec_time_ns
    out = np.concatenate([res.results[s]["out"] for s in range(NCORES)],
                         axis=0)
    return out


# revision 5
# speedup vs baseline: 4.9765x; 4.9765x over previous
"""Trainium2 Bass kernel for nn_BoardEncoder (HexConv board encoder).

Math:
  h[b,n,:] = relu(x[b,n] @ Wc.T + sum_k neighbors[b,n,k] @ Wd[k].T + bc + bd.sum(0))
  out[b]   = h[b].reshape(216) @ Wf.T + bf

Strategy (pure data-parallel over batch, 8 cores x 2048 rows):
  - Host packs per-(b,n) token features [x | neighbors | 1.0] into a
    feature-major bf16 layout xt[n, p, c*2048 + b] = feat[c*114 + p] so the
    device streams big contiguous DMAs and the PE contracts over the
    feature dim (features on partitions), K split into 4 chunks of 114.
    bf16 halves HBM traffic (tolerance is 2e-2; bf16 lands ~3e-3) and runs
    the PE at 1 cycle/row instead of fp32's 4.
  - Input loads are whole-tile [113, 16KB] dma_starts on the sync HWDGE
    ring: HWDGE assigns descriptors round-robin across all 16 SDMA engines
    (~25 GB/s each), so a single queue sustains ~350+ GB/s. (The SWDGE
    gpsimd path concentrates a partition-sliced load on ~3 engines --
    measured 95 GB/s aggregate -- so it is NOT used for the big loads.)
  - Stage 1 (per board cell n): psum[4, 512b] += Wchunk.T @ xtchunk over 4
    chunks, relu (vector/gpsimd alternate) -> bf16 strip [4, 2048], then a
    SBUF->SBUF DMA on the scalar HWDGE ring scatters the strip to partition
    4n of the h^T [(n,h), b] accumulator.
  - Stage 2: out[128b, 256] = hA.T @ WfT[:128] + hB.T @ WfT[128:] with a
    constant ones-row in hB providing the bf bias.
"""

import sys

sys.path.insert(0, "/opt/trn_rl_repo")

import numpy as np

B = 16384
N = 54
D_IN = 64
KN = 6
D_HID = 4
D_OUT = 256
NCORES = 8
BS = B // NCORES          # 2048 batch rows per core
F = D_IN + KN * D_IN + 1  # 449 features incl. constant-1 bias feature
CH = 114                  # K-chunk partition size; EVEN so whole-tile
                          # loads spread over all 16 SDMA engines (odd
                          # partition counts serialize onto one engine)
NCH = 4
FPAD = CH * NCH           # 456
BT = 512                  # stage-1 moving free dim (tokens per matmul)
NBT = BS // BT            # 4

LAST_EXEC_NS = None

_PROGRAM = None


def _build_program(reps=1):
    import concourse.bacc as bacc
    import concourse.tile as tile
    from concourse import mybir

    f32 = mybir.dt.float32
    bf16 = mybir.dt.bfloat16

    nc = bacc.Bacc("TRN2", target_bir_lowering=False, debug=False,
                   num_devices=NCORES)
    xt_d = nc.declare_dram_parameter("xt", [N, CH, NCH * BS], bf16,
                                     isOutput=False)
    w_d = nc.declare_dram_parameter("w", [CH, NCH * D_HID], bf16,
                                    isOutput=False)
    wfta_d = nc.declare_dram_parameter("wfta", [128, D_OUT], bf16,
                                       isOutput=False)
    wftb_d = nc.declare_dram_parameter("wftb", [89, D_OUT], bf16,
                                       isOutput=False)
    out_d = nc.declare_dram_parameter("out", [BS, D_OUT], f32, isOutput=True)

    with tile.TileContext(nc) as tc:
        with (
            tc.tile_pool(name="consts", bufs=1) as consts,
            tc.tile_pool(name="hacc", bufs=1) as hacc,
            tc.tile_pool(name="xt", bufs=6) as xtp,
            tc.tile_pool(name="hn", bufs=3) as hnp,
            tc.tile_pool(name="ps1", bufs=4, space="PSUM") as ps1,
            tc.tile_pool(name="ps2", bufs=2, space="PSUM") as ps2,
            tc.tile_pool(name="outp", bufs=3) as outp,
        ):
            w_sb = consts.tile([CH, NCH * D_HID], bf16, tag="w")
            nc.sync.dma_start(w_sb[:], w_d[:])
            wfta_sb = consts.tile([128, D_OUT], bf16, tag="wfta")
            nc.sync.dma_start(wfta_sb[:], wfta_d[:])
            wftb_sb = consts.tile([89, D_OUT], bf16, tag="wftb")
            nc.sync.dma_start(wftb_sb[:], wftb_d[:])

            for rep in range(reps):
                hA = hacc.tile([128, BS], bf16, tag="hA")  # (n,h) rows 0..127
                hB = hacc.tile([89, BS], bf16, tag="hB")   # rows 128..215+ones
                # rows 0..87 are overwritten by the per-cell scatter DMAs
                # below; row 88 keeps the 1.0 fill and provides the bf bias
                # in stage 2. (a [88:89] memset is rejected: compute-engine
                # partition bases must be 32-aligned)
                nc.gpsimd.memset(hB[:, :], 1.0)

                for n in range(N):
                    xt = xtp.tile([CH, NCH * BS], bf16)
                    # whole-tile load on the sync HWDGE ring: 113 descriptors
                    # of 16 KB round-robin across all 16 SDMA engines
                    nc.sync.dma_start(xt[:], xt_d[n])
                    hn = hnp.tile([D_HID, BS], bf16)
                    for bt in range(NBT):
                        ps = ps1.tile([D_HID, BT], f32)
                        for c in range(NCH):
                            nc.tensor.matmul(
                                ps[:],
                                w_sb[:, c * D_HID:(c + 1) * D_HID],
                                xt[:, c * BS + bt * BT:
                                   c * BS + (bt + 1) * BT],
                                start=(c == 0),
                                stop=(c == NCH - 1),
                            )
                        dst = hn[:, bt * BT:(bt + 1) * BT]
                        if (n * NBT + bt) % 2 == 0:
                            nc.vector.tensor_scalar_max(dst, ps[:], 0.0)
                        else:
                            nc.scalar.activation(
                                dst, ps[:],
                                mybir.ActivationFunctionType.Relu)
                    # scatter on gpsimd/SWDGE: small (16 KB) and keeps its
                    # relu sem-wait off both HWDGE rings (loads on sync,
                    # stores on scalar)
                    if n < 32:
                        nc.gpsimd.dma_start(hA[n * 4:(n + 1) * 4, :], hn[:])
                    else:
                        m = n - 32
                        nc.gpsimd.dma_start(hB[m * 4:(m + 1) * 4, :], hn[:])

                for t in range(BS // 128):
                    po = ps2.tile([128, D_OUT], f32)
                    nc.tensor.matmul(po[:], hA[:, t * 128:(t + 1) * 128],
                                     wfta_sb[:], start=True, stop=False)
                    nc.tensor.matmul(po[:], hB[:, t * 128:(t + 1) * 128],
                                     wftb_sb[:], start=False, stop=True)
                    ot = outp.tile([128, D_OUT], f32)
                    if t % 2 == 0:
                        nc.vector.tensor_copy(ot[:], po[:])
                    else:
                        nc.scalar.copy(ot[:], po[:])
                    nc.scalar.dma_start(out_d[t * 128:(t + 1) * 128, :], ot[:])

    nc.compile()
    return nc


def _get_program():
    global _PROGRAM
    if _PROGRAM is None:
        _PROGRAM = _build_program()
    return _PROGRAM


def _pack_inputs(x, neighbors):
    """Per-shard feature-major packing: xt[n, p, c*BS + b] = feat[c*114+p]
    of batch row (shard*BS + b), cell n. feat = [x | neighbors | 1 | 0pad],
    cast to bf16."""
    import ml_dtypes

    bf16 = np.dtype(ml_dtypes.bfloat16)
    xts = []
    feat = np.empty((BS, N, FPAD), np.float32)
    feat[:, :, F - 1] = 1.0
    feat[:, :, F:] = 0.0
    for s in range(NCORES):
        sl = slice(s * BS, (s + 1) * BS)
        feat[:, :, :D_IN] = x[sl]
        feat[:, :, D_IN:F - 1] = neighbors[sl].reshape(BS, N, KN * D_IN)
        fb = feat.astype(bf16)                       # [BS, N, 452]
        fv = fb.reshape(BS, N, NCH, CH)
        xt = np.ascontiguousarray(fv.transpose(1, 3, 2, 0)).reshape(
            N, CH, NCH * BS)
        xts.append(xt)
    return xts


def _pack_weights(Wc, bc, Wd, bd, Wf, bf):
    import ml_dtypes

    bf16 = np.dtype(ml_dtypes.bfloat16)
    W_all = np.zeros((FPAD, D_HID), np.float32)
    W_all[:D_IN] = Wc.T
    W_all[D_IN:F - 1] = Wd.transpose(0, 2, 1).reshape(KN * D_IN, D_HID)
    W_all[F - 1] = bc + bd.sum(0)
    # w[p, c*4+h] = W_all[c*114+p, h]
    w = np.ascontiguousarray(
        W_all.reshape(NCH, CH, D_HID).transpose(1, 0, 2)).reshape(
            CH, NCH * D_HID).astype(bf16)
    WfT = np.ascontiguousarray(Wf.T)            # [216, 256]
    wfta = np.ascontiguousarray(WfT[:128]).astype(bf16)
    wftb = np.concatenate([WfT[128:], bf[None, :]], axis=0)  # [89, 256]
    wftb = np.ascontiguousarray(wftb).astype(bf16)
    return w, wfta, wftb


def kernel(x, neighbors, Wc, bc, Wd, bd, Wf, bf):
    global LAST_EXEC_NS
    from concourse.bass_utils import run_bass_kernel_spmd

    x = np.asarray(x, np.float32)
    neighbors = np.asarray(neighbors, np.float32)
    w, wfta, wftb = _pack_weights(
        np.asarray(Wc, np.float32), np.asarray(bc, np.float32),
        np.asarray(Wd, np.float32), np.asarray(bd, np.float32),
        np.asarray(Wf, np.float32), np.asarray(bf, np.float32))
    xts = _pack_inputs(x, neighbors)

    nc = _get_program()
    in_maps = [
        {"xt": xts[s], "w": w, "wfta": wfta, "wftb": wftb}
        for s in range(NCORES)
    ]
    res = run_bass_kernel_spmd(nc, in_maps, list(range(NCORES)))
    LAST_EXEC_NS = res.exec_time_ns
    out = np.concatenate([res.results[s]["out"] for s in range(NCORES)],
                         axis=0)
    return out


# revision 6
# speedup vs baseline: 9.7944x; 1.9681x over previous
"""Trainium2 Bass kernel for nn_BoardEncoder (HexConv board encoder).

Math:
  h[b,n,:] = relu(x[b,n] @ Wc.T + sum_k neighbors[b,n,k] @ Wd[k].T + bc + bd.sum(0))
  out[b]   = h[b].reshape(216) @ Wf.T + bf

Strategy (pure data-parallel over batch, 8 cores x 2048 rows):
  - Host packs per-(b,n) token features [x | neighbors] (448 of them) into a
    feature-major bf16 layout split into two tiles per cell: xtA[n, p, c*2048+b]
    = feat[c*128+p] (chunks 0-2, 128 features each) and xtB[n, p, b] =
    feat[384+p] (chunk 3, 64 features). bf16 halves HBM traffic (tolerance is
    2e-2; bf16 lands ~3e-3) and runs the PE at 1 cycle/row instead of fp32's 4.
  - Partition counts are multiples of 16 because a P-partition DMA is split
    over k = (largest divisor of P <= 16) SDMA engines: P=128/64 engages all
    16 engines (~25 GB/s each, ~360-410 GB/s aggregate); odd P serializes
    onto ONE engine. All big loads ride the sync HWDGE ring.
  - Stage 1 (per board cell n): psum[4, 512b] accumulates 4 chunk matmuls,
    then bias+relu fused on vector/scalar (bias = bc + bd.sum(0), applied as
    a per-partition scalar instead of a constant-1 input feature) -> bf16
    strip [4, 2048]; a SBUF->SBUF DMA on gpsimd/SWDGE scatters the strip to
    partition 4n of the h^T [(n,h), b] accumulator.
  - Stage 2: out[128b, 256] = hA.T @ WfT[:128] + hB.T @ WfT[128:] with a
    constant ones-row in hB providing the bf bias.
"""

import sys

sys.path.insert(0, "/opt/trn_rl_repo")

import numpy as np

B = 16384
N = 54
D_IN = 64
KN = 6
D_HID = 4
D_OUT = 256
NCORES = 8
BS = B // NCORES          # 2048 batch rows per core
F = D_IN + KN * D_IN      # 448 features (bias fused into the relu)
CHA = 128                 # chunk height, tile A (3 chunks)
CHB = 64                  # chunk height, tile B (1 chunk)
NCA = 3
BT = 512                  # stage-1 moving free dim (tokens per matmul)
NBT = BS // BT            # 4

LAST_EXEC_NS = None

_PROGRAM = None


def _build_program(reps=1):
    import concourse.bacc as bacc
    import concourse.tile as tile
    from concourse import mybir

    f32 = mybir.dt.float32
    bf16 = mybir.dt.bfloat16
    Alu = mybir.AluOpType

    nc = bacc.Bacc("TRN2", target_bir_lowering=False, debug=False,
                   num_devices=NCORES)
    xta_d = nc.declare_dram_parameter("xta", [N, CHA, NCA * BS], bf16,
                                      isOutput=False)
    xtb_d = nc.declare_dram_parameter("xtb", [N, CHB, BS], bf16,
                                      isOutput=False)
    wa_d = nc.declare_dram_parameter("wa", [CHA, NCA * D_HID], bf16,
                                     isOutput=False)
    wb_d = nc.declare_dram_parameter("wb", [CHB, D_HID], bf16,
                                     isOutput=False)
    bias_d = nc.declare_dram_parameter("biash", [D_HID, 1], f32,
                                       isOutput=False)
    wfta_d = nc.declare_dram_parameter("wfta", [128, D_OUT], bf16,
                                       isOutput=False)
    wftb_d = nc.declare_dram_parameter("wftb", [89, D_OUT], bf16,
                                       isOutput=False)
    out_d = nc.declare_dram_parameter("out", [BS, D_OUT], f32, isOutput=True)

    with tile.TileContext(nc) as tc:
        with (
            tc.tile_pool(name="consts", bufs=1) as consts,
            tc.tile_pool(name="hacc", bufs=1) as hacc,
            tc.tile_pool(name="xta", bufs=6) as xtap,
            tc.tile_pool(name="xtb", bufs=6) as xtbp,
            tc.tile_pool(name="hn", bufs=3) as hnp,
            tc.tile_pool(name="ps1", bufs=4, space="PSUM") as ps1,
            tc.tile_pool(name="ps2", bufs=2, space="PSUM") as ps2,
            tc.tile_pool(name="outp", bufs=3) as outp,
        ):
            wa_sb = consts.tile([CHA, NCA * D_HID], bf16, tag="wa")
            nc.sync.dma_start(wa_sb[:], wa_d[:])
            wb_sb = consts.tile([CHB, D_HID], bf16, tag="wb")
            nc.sync.dma_start(wb_sb[:], wb_d[:])
            bias_sb = consts.tile([D_HID, 1], f32, tag="biash")
            nc.sync.dma_start(bias_sb[:], bias_d[:])
            wfta_sb = consts.tile([128, D_OUT], bf16, tag="wfta")
            nc.sync.dma_start(wfta_sb[:], wfta_d[:])
            wftb_sb = consts.tile([89, D_OUT], bf16, tag="wftb")
            nc.sync.dma_start(wftb_sb[:], wftb_d[:])

            for rep in range(reps):
                hA = hacc.tile([128, BS], bf16, tag="hA")  # (n,h) rows 0..127
                hB = hacc.tile([89, BS], bf16, tag="hB")   # rows 128..215+ones
                # rows 0..87 are overwritten by the per-cell scatter DMAs
                # below; row 88 keeps the 1.0 fill and provides the bf bias
                # in stage 2. (a [88:89] memset is rejected: compute-engine
                # partition bases must be 32-aligned)
                nc.gpsimd.memset(hB[:, :], 1.0)

                for n in range(N):
                    xta = xtap.tile([CHA, NCA * BS], bf16)
                    nc.sync.dma_start(xta[:], xta_d[n])
                    xtb = xtbp.tile([CHB, BS], bf16)
                    nc.sync.dma_start(xtb[:], xtb_d[n])
                    hn = hnp.tile([D_HID, BS], bf16)
                    for bt in range(NBT):
                        ps = ps1.tile([D_HID, BT], f32)
                        for c in range(NCA):
                            nc.tensor.matmul(
                                ps[:],
                                wa_sb[:, c * D_HID:(c + 1) * D_HID],
                                xta[:, c * BS + bt * BT:
                                    c * BS + (bt + 1) * BT],
                                start=(c == 0),
                                stop=False,
                            )
                        nc.tensor.matmul(
                            ps[:], wb_sb[:],
                            xtb[:, bt * BT:(bt + 1) * BT],
                            start=False, stop=True,
                        )
                        dst = hn[:, bt * BT:(bt + 1) * BT]
                        if (n * NBT + bt) % 2 == 0:
                            # max(ps + bias, 0)
                            nc.vector.tensor_scalar(
                                dst, ps[:], bias_sb[:, 0:1], 0.0,
                                Alu.add, Alu.max)
                        else:
                            nc.scalar.activation(
                                dst, ps[:],
                                mybir.ActivationFunctionType.Relu,
                                bias=bias_sb[:, 0:1])
                    # scatter on gpsimd/SWDGE: small (16 KB) and keeps its
                    # relu sem-wait off both HWDGE rings (loads on sync,
                    # stores on scalar)
                    if n < 32:
                        nc.gpsimd.dma_start(hA[n * 4:(n + 1) * 4, :], hn[:])
                    else:
                        m = n - 32
                        nc.gpsimd.dma_start(hB[m * 4:(m + 1) * 4, :], hn[:])

                for t in range(BS // 128):
                    po = ps2.tile([128, D_OUT], f32)
                    nc.tensor.matmul(po[:], hA[:, t * 128:(t + 1) * 128],
                                     wfta_sb[:], start=True, stop=False)
                    nc.tensor.matmul(po[:], hB[:, t * 128:(t + 1) * 128],
                                     wftb_sb[:], start=False, stop=True)
                    ot = outp.tile([128, D_OUT], f32)
                    if t % 2 == 0:
                        nc.vector.tensor_copy(ot[:], po[:])
                    else:
                        nc.scalar.copy(ot[:], po[:])
                    nc.scalar.dma_start(out_d[t * 128:(t + 1) * 128, :], ot[:])

    nc.compile()
    return nc


def _get_program():
    global _PROGRAM
    if _PROGRAM is None:
        _PROGRAM = _build_program()
    return _PROGRAM


def _pack_inputs(x, neighbors):
    """Per-shard feature-major packing, bf16:
    xtA[n, p, c*BS + b] = feat[c*128 + p] (c = 0..2),
    xtB[n, p, b] = feat[384 + p], feat = [x | neighbors] (448)."""
    import ml_dtypes

    bf16 = np.dtype(ml_dtypes.bfloat16)
    xtas, xtbs = [], []
    feat = np.empty((BS, N, F), np.float32)
    for s in range(NCORES):
        sl = slice(s * BS, (s + 1) * BS)
        feat[:, :, :D_IN] = x[sl]
        feat[:, :, D_IN:] = neighbors[sl].reshape(BS, N, KN * D_IN)
        fb = feat.astype(bf16)                       # [BS, N, 448]
        fa = fb[:, :, :NCA * CHA].reshape(BS, N, NCA, CHA)
        xta = np.ascontiguousarray(fa.transpose(1, 3, 2, 0)).reshape(
            N, CHA, NCA * BS)
        xtb = np.ascontiguousarray(
            fb[:, :, NCA * CHA:].transpose(1, 2, 0))     # [N, 64, BS]
        xtas.append(xta)
        xtbs.append(xtb)
    return xtas, xtbs


def _pack_weights(Wc, bc, Wd, bd, Wf, bf):
    import ml_dtypes

    bf16 = np.dtype(ml_dtypes.bfloat16)
    W_all = np.empty((F, D_HID), np.float32)
    W_all[:D_IN] = Wc.T
    W_all[D_IN:] = Wd.transpose(0, 2, 1).reshape(KN * D_IN, D_HID)
    # wa[p, c*4+h] = W_all[c*128+p, h]
    wa = np.ascontiguousarray(
        W_all[:NCA * CHA].reshape(NCA, CHA, D_HID).transpose(1, 0, 2)
    ).reshape(CHA, NCA * D_HID).astype(bf16)
    wb = np.ascontiguousarray(W_all[NCA * CHA:]).astype(bf16)   # [64, 4]
    biash = np.ascontiguousarray(
        (bc + bd.sum(0)).reshape(D_HID, 1).astype(np.float32))
    WfT = np.ascontiguousarray(Wf.T)            # [216, 256]
    wfta = np.ascontiguousarray(WfT[:128]).astype(bf16)
    wftb = np.concatenate([WfT[128:], bf[None, :]], axis=0)  # [89, 256]
    wftb = np.ascontiguousarray(wftb).astype(bf16)
    return wa, wb, biash, wfta, wftb


def kernel(x, neighbors, Wc, bc, Wd, bd, Wf, bf):
    global LAST_EXEC_NS
    from concourse.bass_utils import run_bass_kernel_spmd

    x = np.asarray(x, np.float32)
    neighbors = np.asarray(neighbors, np.float32)
    wa, wb, biash, wfta, wftb = _pack_weights(
        np.asarray(Wc, np.float32), np.asarray(bc, np.float32),
        np.asarray(Wd, np.float32), np.asarray(bd, np.float32),
        np.asarray(Wf, np.float32), np.asarray(bf, np.float32))
    xtas, xtbs = _pack_inputs(x, neighbors)

    nc = _get_program()
    in_maps = [
        {"xta": xtas[s], "xtb": xtbs[s], "wa": wa, "wb": wb, "biash": biash,
         "wfta": wfta, "wftb": wftb}
        for s in range(NCORES)
    ]
    res = run_bass_kernel_spmd(nc, in_maps, list(range(NCORES)))
    LAST_EXEC_NS = res.exec_time_ns
    out = np.concatenate([res.results[s]["out"] for s in range(NCORES)],
                         axis=0)
    return out


# revision 7
# speedup vs baseline: 10.6110x; 1.0834x over previous
"""Trainium2 Bass kernel for nn_BoardEncoder (HexConv board encoder).

Math:
  h[b,n,:] = relu(x[b,n] @ Wc.T + sum_k neighbors[b,n,k] @ Wd[k].T + bc + bd.sum(0))
  out[b]   = h[b].reshape(216) @ Wf.T + bf

Strategy (pure data-parallel over batch, 8 cores x 2048 rows):
  - Host packs per-(b,n) token features [x | neighbors] (448 of them) into a
    feature-major bf16 layout: xtA[n, p, c*2048+b] = feat[c*128+p] (chunks
    0-2, 128 features each, one [128, 12KB] load per cell) and
    xtB[n//6, p, (n%6)*2048+b] = feat[384+p] (chunk 3, 64 features, one
    [64, 24KB] load per 6 cells). bf16 halves HBM traffic (tolerance is
    2e-2; bf16 lands ~3e-3) and runs the PE at 1 cycle/row, 4x fp32.
  - Partition counts are multiples of 16 because a P-partition DMA is split
    over k = (largest divisor of P <= 16) SDMA engines: P=128/64 engages all
    16 engines (~25 GB/s each); odd P serializes onto ONE engine. All big
    loads ride the sync HWDGE ring with multi-KB descriptors.
  - Stage 1 processes cells in PAIRS with PE column tiling: cell 2g at
    columns 0-3 (tile_position (0,0)), cell 2g+1 at columns 32-35
    (tile_position (0,32)); the two cells' chunk matmuls run concurrently
    in separate 32-column PE groups. psum[36, 512b] accumulates 4 chunk
    matmuls per cell, then ONE fused bias+relu op covers both cells
    (bias = bc + bd.sum(0) as a per-partition scalar; rows 4-31 are
    garbage and ignored). SBUF->SBUF DMAs on gpsimd/SWDGE scatter rows
    0-3 / 32-35 to partition 4n of the h^T [(n,h), b] accumulator.
  - Stage 2: out[128b, 256] = hA.T @ WfT[:128] + hB.T @ WfT[128:] with a
    constant ones-row in hB providing the bf bias; stores alternate the
    sync/scalar HWDGE rings.
"""

import sys

sys.path.insert(0, "/opt/trn_rl_repo")

import numpy as np

B = 16384
N = 54
D_IN = 64
KN = 6
D_HID = 4
D_OUT = 256
NCORES = 8
BS = B // NCORES          # 2048 batch rows per core
F = D_IN + KN * D_IN      # 448 features (bias fused into the relu)
CHA = 128                 # chunk height, tile A (3 chunks)
CHB = 64                  # chunk height, tile B (1 chunk)
NCA = 3
GB = 6                    # cells per xtB load group
BT = 512                  # stage-1 moving free dim (tokens per matmul)
NBT = BS // BT            # 4

LAST_EXEC_NS = None

_PROGRAM = None


def _build_program(reps=1):
    import concourse.bacc as bacc
    import concourse.tile as tile
    from concourse import mybir

    f32 = mybir.dt.float32
    bf16 = mybir.dt.bfloat16
    Alu = mybir.AluOpType

    nc = bacc.Bacc("TRN2", target_bir_lowering=False, debug=False,
                   num_devices=NCORES)
    xta_d = nc.declare_dram_parameter("xta", [N, CHA, NCA * BS], bf16,
                                      isOutput=False)
    xtb_d = nc.declare_dram_parameter("xtb", [N // GB, CHB, GB * BS], bf16,
                                      isOutput=False)
    wa_d = nc.declare_dram_parameter("wa", [CHA, NCA * D_HID], bf16,
                                     isOutput=False)
    wb_d = nc.declare_dram_parameter("wb", [CHB, D_HID], bf16,
                                     isOutput=False)
    bias_d = nc.declare_dram_parameter("biash", [36, 1], f32,
                                       isOutput=False)
    wfta_d = nc.declare_dram_parameter("wfta", [128, D_OUT], bf16,
                                       isOutput=False)
    wftb_d = nc.declare_dram_parameter("wftb", [89, D_OUT], bf16,
                                       isOutput=False)
    out_d = nc.declare_dram_parameter("out", [BS, D_OUT], f32, isOutput=True)

    with tile.TileContext(nc) as tc:
        with (
            tc.tile_pool(name="consts", bufs=1) as consts,
            tc.tile_pool(name="hacc", bufs=1) as hacc,
            tc.tile_pool(name="xta", bufs=6) as xtap,
            tc.tile_pool(name="xtb", bufs=2) as xtbp,
            tc.tile_pool(name="hn", bufs=3) as hnp,
            tc.tile_pool(name="ps1", bufs=4, space="PSUM") as ps1,
            tc.tile_pool(name="ps2", bufs=2, space="PSUM") as ps2,
            tc.tile_pool(name="outp", bufs=3) as outp,
        ):
            wa_sb = consts.tile([CHA, NCA * D_HID], bf16, tag="wa")
            nc.sync.dma_start(wa_sb[:], wa_d[:])
            wb_sb = consts.tile([CHB, D_HID], bf16, tag="wb")
            nc.sync.dma_start(wb_sb[:], wb_d[:])
            bias_sb = consts.tile([36, 1], f32, tag="biash")
            nc.sync.dma_start(bias_sb[:], bias_d[:])
            wfta_sb = consts.tile([128, D_OUT], bf16, tag="wfta")
            nc.sync.dma_start(wfta_sb[:], wfta_d[:])
            wftb_sb = consts.tile([89, D_OUT], bf16, tag="wftb")
            nc.sync.dma_start(wftb_sb[:], wftb_d[:])

            for rep in range(reps):
                hA = hacc.tile([128, BS], bf16, tag="hA")  # (n,h) rows 0..127
                hB = hacc.tile([89, BS], bf16, tag="hB")   # rows 128..215+ones
                # rows 0..87 are overwritten by the per-cell scatter DMAs
                # below; row 88 keeps the 1.0 fill and provides the bf bias
                # in stage 2. (a [88:89] memset is rejected: compute-engine
                # partition bases must be 32-aligned)
                nc.gpsimd.memset(hB[:, :], 1.0)

                def scatter(n, hn, j):
                    src = hn[32 * j:32 * j + D_HID, :]
                    if n < 32:
                        nc.gpsimd.dma_start(hA[n * 4:(n + 1) * 4, :], src)
                    else:
                        m = n - 32
                        nc.gpsimd.dma_start(hB[m * 4:(m + 1) * 4, :], src)

                xtb = None
                for g in range(N // 2):
                    n0 = 2 * g
                    if n0 % GB == 0:
                        xtb = xtbp.tile([CHB, GB * BS], bf16)
                        nc.sync.dma_start(xtb[:], xtb_d[n0 // GB])
                    xtas = []
                    for j in range(2):
                        xta = xtap.tile([CHA, NCA * BS], bf16)
                        nc.sync.dma_start(xta[:], xta_d[n0 + j])
                        xtas.append(xta)
                    hn = hnp.tile([36, BS], bf16)
                    for bt in range(NBT):
                        ps = ps1.tile([36, BT], f32)
                        for c in range(NCA):
                            for j in range(2):
                                nc.tensor.matmul(
                                    ps[32 * j:32 * j + D_HID, :],
                                    wa_sb[:, c * D_HID:(c + 1) * D_HID],
                                    xtas[j][:, c * BS + bt * BT:
                                            c * BS + (bt + 1) * BT],
                                    start=(c == 0),
                                    stop=False,
                                    tile_position=(0, 32 * j),
                                )
                        for j in range(2):
                            off = (n0 + j) % GB
                            nc.tensor.matmul(
                                ps[32 * j:32 * j + D_HID, :], wb_sb[:],
                                xtb[:, off * BS + bt * BT:
                                    off * BS + (bt + 1) * BT],
                                start=False, stop=True,
                                tile_position=(0, 32 * j),
                            )
                        # one fused bias+relu over both cells' psum rows
                        dst = hn[:, bt * BT:(bt + 1) * BT]
                        if (g * NBT + bt) % 2 == 0:
                            nc.vector.tensor_scalar(
                                dst, ps[:], bias_sb[:, 0:1], 0.0,
                                Alu.add, Alu.max)
                        else:
                            nc.scalar.activation(
                                dst, ps[:],
                                mybir.ActivationFunctionType.Relu,
                                bias=bias_sb[:, 0:1])
                    scatter(n0, hn, 0)
                    scatter(n0 + 1, hn, 1)

                for t in range(BS // 128):
                    po = ps2.tile([128, D_OUT], f32)
                    nc.tensor.matmul(po[:], hA[:, t * 128:(t + 1) * 128],
                                     wfta_sb[:], start=True, stop=False)
                    nc.tensor.matmul(po[:], hB[:, t * 128:(t + 1) * 128],
                                     wftb_sb[:], start=False, stop=True)
                    ot = outp.tile([128, D_OUT], f32)
                    if t % 2 == 0:
                        nc.vector.tensor_copy(ot[:], po[:])
                    else:
                        nc.scalar.copy(ot[:], po[:])
                    eng = nc.sync if t % 2 == 0 else nc.scalar
                    eng.dma_start(out_d[t * 128:(t + 1) * 128, :], ot[:])

    nc.compile()
    return nc


def _get_program():
    global _PROGRAM
    if _PROGRAM is None:
        _PROGRAM = _build_program()
    return _PROGRAM


def _pack_inputs(x, neighbors):
    """Per-shard feature-major packing, bf16:
    xtA[n, p, c*BS + b] = feat[c*128 + p] (c = 0..2),
    xtB[n//GB, p, (n%GB)*BS + b] = feat[384 + p], feat = [x | neighbors]."""
    import ml_dtypes

    bf16 = np.dtype(ml_dtypes.bfloat16)
    xtas, xtbs = [], []
    feat = np.empty((BS, N, F), np.float32)
    for s in range(NCORES):
        sl = slice(s * BS, (s + 1) * BS)
        feat[:, :, :D_IN] = x[sl]
        feat[:, :, D_IN:] = neighbors[sl].reshape(BS, N, KN * D_IN)
        fb = feat.astype(bf16)                       # [BS, N, 448]
        fa = fb[:, :, :NCA * CHA].reshape(BS, N, NCA, CHA)
        xta = np.ascontiguousarray(fa.transpose(1, 3, 2, 0)).reshape(
            N, CHA, NCA * BS)
        # [BS, N, 64] -> [N//GB, 64, GB*BS]
        fbb = fb[:, :, NCA * CHA:].reshape(BS, N // GB, GB, CHB)
        xtb = np.ascontiguousarray(fbb.transpose(1, 3, 2, 0)).reshape(
            N // GB, CHB, GB * BS)
        xtas.append(xta)
        xtbs.append(xtb)
    return xtas, xtbs


def _pack_weights(Wc, bc, Wd, bd, Wf, bf):
    import ml_dtypes

    bf16 = np.dtype(ml_dtypes.bfloat16)
    W_all = np.empty((F, D_HID), np.float32)
    W_all[:D_IN] = Wc.T
    W_all[D_IN:] = Wd.transpose(0, 2, 1).reshape(KN * D_IN, D_HID)
    # wa[p, c*4+h] = W_all[c*128+p, h]
    wa = np.ascontiguousarray(
        W_all[:NCA * CHA].reshape(NCA, CHA, D_HID).transpose(1, 0, 2)
    ).reshape(CHA, NCA * D_HID).astype(bf16)
    wb = np.ascontiguousarray(W_all[NCA * CHA:]).astype(bf16)   # [64, 4]
    bias_h = (bc + bd.sum(0)).astype(np.float32)
    biash = np.zeros((36, 1), np.float32)
    biash[0:D_HID, 0] = bias_h
    biash[32:32 + D_HID, 0] = bias_h
    WfT = np.ascontiguousarray(Wf.T)            # [216, 256]
    wfta = np.ascontiguousarray(WfT[:128]).astype(bf16)
    wftb = np.concatenate([WfT[128:], bf[None, :]], axis=0)  # [89, 256]
    wftb = np.ascontiguousarray(wftb).astype(bf16)
    return wa, wb, biash, wfta, wftb


def kernel(x, neighbors, Wc, bc, Wd, bd, Wf, bf):
    global LAST_EXEC_NS
    from concourse.bass_utils import run_bass_kernel_spmd

    x = np.asarray(x, np.float32)
    neighbors = np.asarray(neighbors, np.float32)
    wa, wb, biash, wfta, wftb = _pack_weights(
        np.asarray(Wc, np.float32), np.asarray(bc, np.float32),
        np.asarray(Wd, np.float32), np.asarray(bd, np.float32),
        np.asarray(Wf, np.float32), np.asarray(bf, np.float32))
    xtas, xtbs = _pack_inputs(x, neighbors)

    nc = _get_program()
    in_maps = [
        {"xta": xtas[s], "xtb": xtbs[s], "wa": wa, "wb": wb, "biash": biash,
         "wfta": wfta, "wftb": wftb}
        for s in range(NCORES)
    ]
    res = run_bass_kernel_spmd(nc, in_maps, list(range(NCORES)))
    LAST_EXEC_NS = res.exec_time_ns
    out = np.concatenate([res.results[s]["out"] for s in range(NCORES)],
                         axis=0)
    return out


# revision 9
# speedup vs baseline: 12.8838x; 1.2142x over previous
"""Trainium2 Bass kernel for nn_BoardEncoder (HexConv board encoder).

Math:
  h[b,n,:] = relu(x[b,n] @ Wc.T + sum_k neighbors[b,n,k] @ Wd[k].T + bc + bd.sum(0))
  out[b]   = h[b].reshape(216) @ Wf.T + bf

Strategy (pure data-parallel over batch, 8 cores x 2048 rows):
  - Host packs per-(b,n) token features [x | neighbors] (448 of them) into a
    feature-major bf16 layout, ONE [128, 14KB] load per cell: columns
    0..3*BS hold chunks 0-2 (feat[c*128+p]); columns 3*BS..3.5*BS hold
    chunk 3 (64 features) with the two token halves stacked on partition
    halves (p<64: tokens 0..1023, p>=64: tokens 1024..2047). bf16 halves
    HBM traffic (tolerance 2e-2; bf16 lands ~3e-3), PE runs 1 cycle/row.
  - Loads are always FULL 128-partition transfers: a P-partition DMA is
    split over k = (largest divisor of P <= 16) SDMA engines, and partial-
    partition transfers also run at half rate per descriptor (measured).
    All big loads ride the sync HWDGE ring with 8-16KB descriptors.
  - Stage 1 processes cells in PAIRS with PE column tiling: cell 2g at
    columns 0-3 (tile_position (0,0)), cell 2g+1 at columns 32-35
    (tile_position (0,32)); the two cells' chunk matmuls run concurrently
    in separate 32-column PE groups. psum[36, 512b] accumulates 4 chunk
    matmuls per cell, then ONE fused bias+relu op covers both cells
    (bias = bc + bd.sum(0) as a per-partition scalar; rows 4-31 are
    garbage and ignored). SBUF->SBUF DMAs on gpsimd/SWDGE scatter rows
    0-3 / 32-35 to partition 4n of the h^T [(n,h), b] accumulator.
  - Stage 2: out[128b, 256] = hA.T @ WfT[:128] + hB.T @ WfT[128:] with a
    constant ones-row in hB providing the bf bias; stores alternate the
    sync/scalar HWDGE rings.
"""

import sys

sys.path.insert(0, "/opt/trn_rl_repo")

import numpy as np

B = 16384
N = 54
D_IN = 64
KN = 6
D_HID = 4
D_OUT = 256
NCORES = 8
BS = B // NCORES          # 2048 batch rows per core
F = D_IN + KN * D_IN      # 448 features (bias fused into the relu)
CHA = 128                 # chunk height, tile A (3 chunks)
CHB = 64                  # chunk height, tile B (1 chunk)
NCA = 3
BT = 512                  # stage-1 moving free dim (tokens per matmul)
NBT = BS // BT            # 4

LAST_EXEC_NS = None

_PROGRAM = None


def _build_program(reps=1):
    import concourse.bacc as bacc
    import concourse.tile as tile
    from concourse import mybir

    f32 = mybir.dt.float32
    bf16 = mybir.dt.bfloat16
    Alu = mybir.AluOpType

    nc = bacc.Bacc("TRN2", target_bir_lowering=False, debug=False,
                   num_devices=NCORES)
    xta_d = nc.declare_dram_parameter("xta", [N, CHA, NCA * BS + BS // 2],
                                      bf16, isOutput=False)
    wa_d = nc.declare_dram_parameter("wa", [CHA, NCA * D_HID], bf16,
                                     isOutput=False)
    wb_d = nc.declare_dram_parameter("wb", [128, D_HID], bf16,
                                     isOutput=False)
    bias_d = nc.declare_dram_parameter("biash", [36, 1], f32,
                                       isOutput=False)
    wfta_d = nc.declare_dram_parameter("wfta", [128, D_OUT], bf16,
                                       isOutput=False)
    wftb_d = nc.declare_dram_parameter("wftb", [89, D_OUT], bf16,
                                       isOutput=False)
    out_d = nc.declare_dram_parameter("out", [BS, D_OUT], f32, isOutput=True)

    with tile.TileContext(nc) as tc:
        with (
            tc.tile_pool(name="consts", bufs=1) as consts,
            tc.tile_pool(name="hacc", bufs=1) as hacc,
            tc.tile_pool(name="xta", bufs=6) as xtap,
            tc.tile_pool(name="hn", bufs=3) as hnp,
            tc.tile_pool(name="ps1", bufs=4, space="PSUM") as ps1,
            tc.tile_pool(name="ps2", bufs=2, space="PSUM") as ps2,
            tc.tile_pool(name="outp", bufs=3) as outp,
        ):
            wa_sb = consts.tile([CHA, NCA * D_HID], bf16, tag="wa")
            nc.sync.dma_start(wa_sb[:], wa_d[:])
            wb_sb = consts.tile([128, D_HID], bf16, tag="wb")
            nc.sync.dma_start(wb_sb[:], wb_d[:])
            bias_sb = consts.tile([36, 1], f32, tag="biash")
            nc.sync.dma_start(bias_sb[:], bias_d[:])
            wfta_sb = consts.tile([128, D_OUT], bf16, tag="wfta")
            nc.sync.dma_start(wfta_sb[:], wfta_d[:])
            wftb_sb = consts.tile([89, D_OUT], bf16, tag="wftb")
            nc.sync.dma_start(wftb_sb[:], wftb_d[:])

            for rep in range(reps):
                hA = hacc.tile([128, BS], bf16, tag="hA")  # (n,h) rows 0..127
                hB = hacc.tile([89, BS], bf16, tag="hB")   # rows 128..215+ones
                # rows 0..87 are overwritten by the per-cell scatter DMAs
                # below; row 88 keeps the 1.0 fill and provides the bf bias
                # in stage 2. (a [88:89] memset is rejected: compute-engine
                # partition bases must be 32-aligned)
                nc.gpsimd.memset(hB[:, :], 1.0)

                def scatter(n, hn, j):
                    src = hn[32 * j:32 * j + D_HID, :]
                    if n < 32:
                        nc.gpsimd.dma_start(hA[n * 4:(n + 1) * 4, :], src)
                    else:
                        m = n - 32
                        nc.gpsimd.dma_start(hB[m * 4:(m + 1) * 4, :], src)

                for g in range(N // 2):
                    n0 = 2 * g
                    xtas = []
                    for j in range(2):
                        xta = xtap.tile([CHA, NCA * BS + BS // 2], bf16)
                        nc.sync.dma_start(xta[:], xta_d[n0 + j])
                        xtas.append(xta)
                    hn = hnp.tile([36, BS], bf16)
                    for bt in range(NBT):
                        ps = ps1.tile([36, BT], f32)
                        for c in range(NCA):
                            for j in range(2):
                                nc.tensor.matmul(
                                    ps[32 * j:32 * j + D_HID, :],
                                    wa_sb[:, c * D_HID:(c + 1) * D_HID],
                                    xtas[j][:, c * BS + bt * BT:
                                            c * BS + (bt + 1) * BT],
                                    start=(c == 0),
                                    stop=False,
                                    tile_position=(0, 32 * j),
                                )
                        # chunk 3: token halves stacked on partition halves
                        pb = 0 if bt < 2 else 64
                        c3 = NCA * BS + (bt % 2) * BT
                        for j in range(2):
                            nc.tensor.matmul(
                                ps[32 * j:32 * j + D_HID, :],
                                wb_sb[pb:pb + CHB, :],
                                xtas[j][pb:pb + CHB, c3:c3 + BT],
                                start=False, stop=True,
                                tile_position=(pb, 32 * j),
                            )
                        # one fused bias+relu over both cells' psum rows
                        dst = hn[:, bt * BT:(bt + 1) * BT]
                        if (g * NBT + bt) % 2 == 0:
                            nc.vector.tensor_scalar(
                                dst, ps[:], bias_sb[:, 0:1], 0.0,
                                Alu.add, Alu.max)
                        else:
                            nc.scalar.activation(
                                dst, ps[:],
                                mybir.ActivationFunctionType.Relu,
                                bias=bias_sb[:, 0:1])
                    scatter(n0, hn, 0)
                    scatter(n0 + 1, hn, 1)

                for t in range(BS // 128):
                    po = ps2.tile([128, D_OUT], f32)
                    nc.tensor.matmul(po[:], hA[:, t * 128:(t + 1) * 128],
                                     wfta_sb[:], start=True, stop=False)
                    nc.tensor.matmul(po[:], hB[:, t * 128:(t + 1) * 128],
                                     wftb_sb[:], start=False, stop=True)
                    ot = outp.tile([128, D_OUT], f32)
                    if t % 2 == 0:
                        nc.vector.tensor_copy(ot[:], po[:])
                    else:
                        nc.scalar.copy(ot[:], po[:])
                    eng = nc.sync if t % 2 == 0 else nc.scalar
                    eng.dma_start(out_d[t * 128:(t + 1) * 128, :], ot[:])

    nc.compile()
    return nc


def _get_program():
    global _PROGRAM
    if _PROGRAM is None:
        _PROGRAM = _build_program()
    return _PROGRAM


def _pack_inputs(x, neighbors):
    """Per-shard feature-major packing, bf16. One [128, 3.5*BS] tile per
    cell: cols 0..3*BS = chunks 0-2 (feat[c*128+p]); cols 3*BS..3.5*BS =
    chunk 3 with token halves stacked on partition halves."""
    import ml_dtypes

    bf16 = np.dtype(ml_dtypes.bfloat16)
    xtas = []
    feat = np.empty((BS, N, F), np.float32)
    for s in range(NCORES):
        sl = slice(s * BS, (s + 1) * BS)
        feat[:, :, :D_IN] = x[sl]
        feat[:, :, D_IN:] = neighbors[sl].reshape(BS, N, KN * D_IN)
        fb = feat.astype(bf16)                       # [BS, N, 448]
        xta = np.empty((N, CHA, NCA * BS + BS // 2), bf16)
        fa = fb[:, :, :NCA * CHA].reshape(BS, N, NCA, CHA)
        xta[:, :, :NCA * BS] = fa.transpose(1, 3, 2, 0).reshape(
            N, CHA, NCA * BS)
        # chunk 3: [BS, N, 64] -> [N, 2, 64, BS//2] -> [N, 128, BS//2]
        fbb = fb[:, :, NCA * CHA:].reshape(2, BS // 2, N, CHB)
        xta[:, :, NCA * BS:] = fbb.transpose(2, 0, 3, 1).reshape(
            N, CHA, BS // 2)
        xtas.append(xta)
    return xtas


def _pack_weights(Wc, bc, Wd, bd, Wf, bf):
    import ml_dtypes

    bf16 = np.dtype(ml_dtypes.bfloat16)
    W_all = np.empty((F, D_HID), np.float32)
    W_all[:D_IN] = Wc.T
    W_all[D_IN:] = Wd.transpose(0, 2, 1).reshape(KN * D_IN, D_HID)
    # wa[p, c*4+h] = W_all[c*128+p, h]
    wa = np.ascontiguousarray(
        W_all[:NCA * CHA].reshape(NCA, CHA, D_HID).transpose(1, 0, 2)
    ).reshape(CHA, NCA * D_HID).astype(bf16)
    # wb duplicated onto both partition halves (chunk-3 token stacking)
    wb = np.concatenate([W_all[NCA * CHA:], W_all[NCA * CHA:]],
                        axis=0).astype(bf16)                    # [128, 4]
    bias_h = (bc + bd.sum(0)).astype(np.float32)
    biash = np.zeros((36, 1), np.float32)
    biash[0:D_HID, 0] = bias_h
    biash[32:32 + D_HID, 0] = bias_h
    WfT = np.ascontiguousarray(Wf.T)            # [216, 256]
    wfta = np.ascontiguousarray(WfT[:128]).astype(bf16)
    wftb = np.concatenate([WfT[128:], bf[None, :]], axis=0)  # [89, 256]
    wftb = np.ascontiguousarray(wftb).astype(bf16)
    return wa, wb, biash, wfta, wftb


def kernel(x, neighbors, Wc, bc, Wd, bd, Wf, bf):
    global LAST_EXEC_NS
    from concourse.bass_utils import run_bass_kernel_spmd

    x = np.asarray(x, np.float32)
    neighbors = np.asarray(neighbors, np.float32)
    wa, wb, biash, wfta, wftb = _pack_weights(
        np.asarray(Wc, np.float32), np.asarray(bc, np.float32),
        np.asarray(Wd, np.float32), np.asarray(bd, np.float32),
        np.asarray(Wf, np.float32), np.asarray(bf, np.float32))
    xtas = _pack_inputs(x, neighbors)

    nc = _get_program()
    in_maps = [
        {"xta": xtas[s], "wa": wa, "wb": wb, "biash": biash,
         "wfta": wfta, "wftb": wftb}
        for s in range(NCORES)
    ]
    res = run_bass_kernel_spmd(nc, in_maps, list(range(NCORES)))
    LAST_EXEC_NS = res.exec_time_ns
    out = np.concatenate([res.results[s]["out"] for s in range(NCORES)],
                         axis=0)
    return out
